# revision 29
# baseline (speedup 1.0000x reference)
"""CharEmb kernel for Trainium2 (8 NeuronCores, batch-sharded).

Computation (per word of 32 chars):
  emb = table[ids]                  # [32 chars, 64] per word
  x[i, j] = emb[i//2, 32*(i%2)+j]   # raw-buffer reshape [64, 32]
  y[f, t] = sum_{i,k} x[i, t+k] * w[f, i, k]   (valid conv, K=3)
  out[f] = max_t y[f, t] + b[f]

Data-parallel: char_ids batch-sharded 2048 words/core; table + conv
weights replicated (inference only, no grads). Pipeline per chunk pair
(64 words, 2048 chars), deeply buffered so the PE stays busy and the
HAM clock-gate spends most of the kernel at 2.4 GHz:
  1. DMA: host-precomputed one-hot [128(pad), 2048] bf16, permuted
     column order (within each 512-col half, col 64*(w//2)+2p+(w%2)
     holds char (w, p)); 128 partitions so the descriptors spread
     round-robin over all 16 SDMA engines (101 rows would serialize
     onto one engine at 27 GB/s).
  2. PE gather, column-group tiled: two M=64 matmuls per chunk at
     tile_position (0,0)/(0,64) run concurrently in disjoint column
     groups of the 128x128 array -> PSUM [128, 512] (char halves
     stacked on partitions).
  3. Act: PSUM -> SBUF bf16 copy [128, 512] per chunk.
  4. DVE: packed 32x32 stream transposes (int32 views of bf16 pairs):
     t_a rows 0:63 = A-data (words 0:15), t_b rows 64:127 = B-data
     (words 16:31, partition-swapped so the tap-2 matmuls can
     row-tile); T[32h+p, 64u+2j+eps] = emb_{w=2u+eps}[p, 32h+j].
  5. DMA shift-dups: the other 64 rows of each tile = data shifted
     +2 bf16 cols (tap k=1 operand).
  6. PE conv per chunk: fused taps 0,1 (128-contraction, stationary
     [W0;W1] for A, [W1;W0] for swapped B) + tap 2 row-tiled: A on
     array rows 0:63 (tile_position (0,0)), B on rows 64:127 ((64,0));
     the two 480-col tap-2 streams overlap in the array.
  7. maxpool over t=30 per chunk: Act compacts the 960 used cols ->
     SBUF bf16; DVE 16-wide overlapping max (covers t0..29, double-
     counting t14/t15 harmlessly) then one 16->1 tensor_reduce.
  8. end: DVE per-partition bias add, one output DMA.
"""

import sys
from contextlib import ExitStack

import numpy as np

if "/opt/trn_rl_repo" not in sys.path:
    sys.path.insert(0, "/opt/trn_rl_repo")

import concourse.bass as bass
import concourse.tile as tile
from concourse import bacc, mybir
from concourse.bass_utils import run_bass_kernel_spmd

# Problem constants (hardcoded per spec)
B, S, C = 32, 512, 32
V, E = 101, 64
F, K = 128, 3
T = C - K + 1  # 30 valid conv positions
NCORES = 8
WORDS = (B * S) // NCORES  # 2048 words per core
NCHARS = WORDS * C  # 65536

CHUNK_W = 32                 # words per chunk
CH_COLS = CHUNK_W * C        # 1024 chars per chunk
NCHUNKS = WORDS // CHUNK_W   # 64
HALF_W = 16                  # words per T-tile half
HALF_COLS = HALF_W * C       # 512

f32 = mybir.dt.float32
bf16 = mybir.dt.bfloat16
i32 = mybir.dt.int32

# maxpool strategy per chunk (cycled): 'a' Act copy + DVE L1+reduce,
# 'd' direct DVE reduce from PSUM
POOL_PATTERN = "a"


def build_kernel(num_devices=NCORES):
    nc = bacc.Bacc(
        "TRN2",
        target_bir_lowering=False,
        debug=False,
        enable_asserts=True,
        num_devices=num_devices,
    )

    oh_d = nc.dram_tensor("oh", [128, NCHARS], bf16, kind="ExternalInput")
    tab_d = nc.dram_tensor("tab", [128, E], bf16, kind="ExternalInput")
    w_d = nc.dram_tensor("wmat", [128, 384], bf16, kind="ExternalInput")
    b_d = nc.dram_tensor("bias", [128, 1], f32, kind="ExternalInput")
    out_d = nc.dram_tensor("out", [128, WORDS], f32, kind="ExternalOutput")

    with tile.TileContext(nc) as tc, ExitStack() as ctx:
        const_pool = ctx.enter_context(tc.tile_pool(name="const", bufs=1))
        oh_pool = ctx.enter_context(tc.tile_pool(name="oh", bufs=8))
        gsb_pool = ctx.enter_context(tc.tile_pool(name="gsb", bufs=6))
        ta_pool = ctx.enter_context(tc.tile_pool(name="tta", bufs=4))
        tb_pool = ctx.enter_context(tc.tile_pool(name="ttb", bufs=4))
        ysb_pool = ctx.enter_context(tc.tile_pool(name="ysb", bufs=4))
        y2_pool = ctx.enter_context(tc.tile_pool(name="y2", bufs=4))
        ob_pool = ctx.enter_context(tc.tile_pool(name="ob", bufs=3))
        g_psum = ctx.enter_context(tc.tile_pool(name="gps", bufs=4, space="PSUM"))
        y_psum = ctx.enter_context(tc.tile_pool(name="yps", bufs=2, space="PSUM"))

        tab_sb = const_pool.tile([128, E], bf16)
        w_sb = const_pool.tile([128, 384], bf16)
        b_sb = const_pool.tile([128, 1], f32)

        nc.sync.dma_start(tab_sb[:], tab_d.ap())
        nc.sync.dma_start(w_sb[:], w_d.ap())
        nc.sync.dma_start(b_sb[:], b_d.ap())

        obt = None
        for pp in range(NCHUNKS // 2):  # chunk pairs
            if pp % 4 == 0:
                # fresh output tile per 8-chunk segment (256 words)
                obt = ob_pool.tile([128, 8 * CHUNK_W], f32)
            # one-hot for the pair, one DMA (128 partitions -> spread)
            oh_t = oh_pool.tile([128, 2 * CH_COLS], bf16)
            nc.sync.dma_start(
                oh_t[:, :],
                oh_d.ap()[:, pp * 2 * CH_COLS:(pp + 1) * 2 * CH_COLS],
            )

            t_a = ta_pool.tile([128, 2 * HALF_COLS], bf16)
            t_b = tb_pool.tile([128, 2 * HALF_COLS], bf16)

            g_list = []
            for cp in range(2):
                # gather matmuls, col-group tiled (concurrent pair)
                g_ps = g_psum.tile([128, HALF_COLS], f32)
                for hh in range(2):
                    nc.tensor.matmul(
                        g_ps[64 * hh:64 * (hh + 1), :],
                        tab_sb[0:128, 0:E],
                        oh_t[0:128, cp * CH_COLS + 512 * hh:
                             cp * CH_COLS + 512 * (hh + 1)],
                        start=True,
                        stop=True,
                        tile_position=(0, 64 * hh),
                    )
                g_list.append(g_ps)

            for cp in range(2):
                g_ps = g_list[cp]
                # PSUM -> SBUF bf16
                gsb = gsb_pool.tile([128, HALF_COLS], bf16)
                nc.scalar.copy(gsb[:, :], g_ps[:, :])
                # packed transposes: A-data -> t_a rows 0:63,
                # B-data -> t_b rows 64:127 (B-swapped layout)
                nc.vector.transpose(
                    t_a[0:64, 512 * cp:512 * (cp + 1)].bitcast(i32),
                    gsb[0:64, :].bitcast(i32),
                )
                nc.vector.transpose(
                    t_b[64:128, 512 * cp:512 * (cp + 1)].bitcast(i32),
                    gsb[64:128, :].bitcast(i32),
                )

            # shift-dups (+2 bf16 cols = +1 j)
            nc.sync.dma_start(
                t_a[64:128, :].rearrange("q (r x) -> q r x", r=2)
                [:, :, 0:510],
                t_a[0:64, :].rearrange("q (r x) -> q r x", r=2)
                [:, :, 2:512],
            )
            nc.sync.dma_start(
                t_b[0:64, :].rearrange("q (r x) -> q r x", r=2)
                [:, :, 0:510],
                t_b[64:128, :].rearrange("q (r x) -> q r x", r=2)
                [:, :, 2:512],
            )

            # conv; taps01 (full array) then row-tiled tap2 A/B pairs
            y_list = []
            v01 = []
            v2 = []
            for cp in range(2):
                y_ps = y_psum.tile([128, 2 * 512], f32)
                y_list.append(y_ps)
                for r, t_t in enumerate((t_a, t_b)):
                    tf = (
                        t_t[:, 512 * cp:512 * (cp + 1)]
                        .rearrange("q (u j e) -> q u e j", j=C, e=2)
                    )
                    if r == 0:
                        tl = (
                            t_a[0:64, 512 * cp:512 * (cp + 1)]
                            .rearrange("q (u j e) -> q u e j", j=C, e=2)
                        )
                        w2 = w_sb[0:64, 128:256]
                        tpos = (0, 0)
                    else:
                        tl = (
                            t_b[64:128, 512 * cp:512 * (cp + 1)]
                            .rearrange("q (u j e) -> q u e j", j=C, e=2)
                        )
                        w2 = w_sb[64:128, 128:256]
                        tpos = (64, 0)
                    out_ap = (
                        y_ps[:, 512 * r:512 * r + HALF_W * T]
                        .rearrange("f (u e t) -> f u e t", t=T, e=2)
                    )
                    w01 = w_sb[:, 0:128] if r == 0 else w_sb[:, 256:384]
                    v01.append((out_ap, w01, tf))
                    v2.append((out_ap, w2, tl, tpos))
            for (out_ap, w01, tf) in v01:
                nc.tensor.matmul(
                    out_ap, w01, tf[:, :, :, 0:T],
                    start=True, stop=False, skip_group_check=True,
                )
            for (out_ap, w2, tl, tpos) in v2:
                nc.tensor.matmul(
                    out_ap, w2, tl[:, :, :, 2:2 + T],
                    start=False, stop=True, skip_group_check=True,
                    tile_position=tpos,
                )

            # maxpool over t -> obuf
            for cp in range(2):
                cc = 2 * pp + cp
                y_ps = y_list[cp]
                strat = POOL_PATTERN[cc % len(POOL_PATTERN)]
                if strat == "d":
                    # direct DVE reduce from PSUM
                    red_in = (
                        y_ps[:, :].rearrange("f (r x) -> f r x", x=512)
                        [:, :, 0:HALF_W * T]
                        .rearrange("f r (w t) -> f r w t", t=T)
                    )
                    nc.vector.tensor_reduce(
                        obt[:, (cc % 8) * CHUNK_W:(cc % 8 + 1) * CHUNK_W]
                        .rearrange("f (r w) -> f r w", w=HALF_W),
                        red_in, axis=mybir.AxisListType.X,
                        op=mybir.AluOpType.max,
                    )
                else:
                    # Act compacts used cols -> SBUF bf16 [128, 960]
                    ysb = ysb_pool.tile([128, 2 * HALF_W * T], bf16)
                    nc.scalar.copy(
                        ysb[:, :].rearrange("f (r y) -> f r y", r=2),
                        y_ps[:, :].rearrange("f (r x) -> f r x", x=512)
                        [:, :, 0:HALF_W * T],
                    )
                    yv = ysb[:, :].rearrange("f (g t) -> f g t", t=T)
                    y2 = y2_pool.tile([128, 32 * 16], bf16)
                    y2v = y2[:, :].rearrange("f (g t) -> f g t", t=16)
                    # L1: 16-wide overlapping max covers t 0..29
                    nc.vector.tensor_tensor(
                        y2v[:, :, :],
                        yv[:, :, 0:16],
                        yv[:, :, 14:30],
                        op=mybir.AluOpType.max,
                    )
                    # L2: 16 -> 8 (2x-rate tensor_tensor)
                    nc.vector.tensor_tensor(
                        y2v[:, :, 0:8],
                        y2v[:, :, 0:8],
                        y2v[:, :, 8:16],
                        op=mybir.AluOpType.max,
                    )
                    # L3: single 8 -> 1 reduce (1x rate, but half the cols)
                    nc.vector.tensor_reduce(
                        obt[:, (cc % 8) * CHUNK_W:(cc % 8 + 1) * CHUNK_W],
                        y2v[:, :, 0:8], axis=mybir.AxisListType.X,
                        op=mybir.AluOpType.max,
                    )
                if cc % 8 == 7:
                    # segment done: bias-add on Act and store, overlapped
                    # with later pairs' compute
                    seg = slice((cc - 7) * CHUNK_W, (cc + 1) * CHUNK_W)
                    nc.scalar.add(obt[:, :], obt[:, :], b_sb[:, 0:1])
                    nc.sync.dma_start(out_d.ap()[:, seg], obt[:, :])



    nc.compile()
    return nc


def host_prep(char_ids, emb_table, conv_w, conv_b, num_devices=NCORES):
    """Build per-core input maps from full inputs."""
    char_ids = np.asarray(char_ids)
    emb_table = np.asarray(emb_table, dtype=np.float32)
    conv_w = np.asarray(conv_w, dtype=np.float32)
    conv_b = np.asarray(conv_b, dtype=np.float32)

    bf = mybir.dt.np(bf16)
    tab = np.zeros((128, E), dtype=np.float32)
    tab[0:V] = emb_table
    tab = tab.astype(bf)

    # permuted ids: within each 512-col half (16 words), column
    # 64*(w'//2) + 2p + (w'%2) holds char (w', p)
    ids_all = char_ids.reshape(-1, HALF_W, C).astype(np.int64)  # [nh, w', p]
    nh = ids_all.shape[0]
    ids_perm = (
        ids_all.reshape(nh, HALF_W // 2, 2, C)
        .transpose(0, 1, 3, 2)  # [half, u, p, eps]
        .reshape(nh * HALF_COLS)
    ).astype(np.int32)

    # one-hot of the permuted ids, zero-padded to 128 rows so the
    # DRAM->SBUF DMA spreads across all 16 SDMA engines
    oh_all = (ids_perm[None, :] == np.arange(128, dtype=np.int32)[:, None])
    oh_all = oh_all.astype(bf)

    # W layout: q = 32h+p (q<64) -> channel 2p+h
    q = np.arange(64)
    ch = 2 * (q % 32) + q // 32
    wmat = np.zeros((128, 384), dtype=np.float32)
    # A stationary [W0; W1]
    wmat[0:64, 0:128] = conv_w[:, ch, 0].T
    wmat[64:128, 0:128] = conv_w[:, ch, 1].T
    # tap2 for A (rows 0:63) and B (rows 64:127)
    wmat[0:64, 128:256] = conv_w[:, ch, 2].T
    wmat[64:128, 128:256] = conv_w[:, ch, 2].T
    # B stationary [W1; W0] (B tile is partition-swapped: dup low, data high)
    wmat[0:64, 256:384] = conv_w[:, ch, 1].T
    wmat[64:128, 256:384] = conv_w[:, ch, 0].T
    wmat = wmat.astype(bf)

    bias = conv_b.reshape(128, 1).astype(np.float32)

    in_maps = []
    for jcore in range(num_devices):
        oh_core = oh_all[:, jcore * NCHARS:(jcore + 1) * NCHARS]
        in_maps.append(
            {
                "oh": np.ascontiguousarray(oh_core),
                "tab": tab,
                "wmat": wmat,
                "bias": bias,
            }
        )
    return in_maps


def _ensure_ntff_hook():
    """The agent image's antenv lacks axon_hooks; shim it and install the
    ctypes NTFF profiling hook so trace=True yields HW exec times."""
    import types

    if "antenv.axon_hooks" in sys.modules:
        return
    mod = types.ModuleType("antenv.axon_hooks")
    _hook = [None]
    mod.get_axon_ntff_profile_hook = lambda: _hook[0]
    mod.set_axon_ntff_profile_hook = lambda h: _hook.__setitem__(0, h)
    sys.modules["antenv.axon_hooks"] = mod
    try:
        import antenv

        antenv.axon_hooks = mod
        from trn_agent_boot.trn_boot import _ntff_profile_via_ctypes

        hook = _ntff_profile_via_ctypes("/opt/axon/libaxon_pjrt.so")
        mod.set_axon_ntff_profile_hook(hook)
    except Exception as e:  # degrade to no-trace
        print(f"ntff hook install failed: {e}", file=sys.stderr)


_NC_CACHE = {}


def _get_nc():
    if "nc" not in _NC_CACHE:
        _NC_CACHE["nc"] = build_kernel()
    return _NC_CACHE["nc"]


def kernel(char_ids, emb_table, conv_w, conv_b, trace=False):
    if trace:
        _ensure_ntff_hook()
    nc = _get_nc()
    in_maps = host_prep(char_ids, emb_table, conv_w, conv_b)
    res = run_bass_kernel_spmd(
        nc, in_maps, core_ids=list(range(NCORES)), trace=trace
    )
    # out[f, word] word-linear -> [word, f]
    outs = [res.results[jc]["out"].T for jc in range(NCORES)]
    full = np.concatenate(outs, axis=0).reshape(B, S, F).astype(np.float32)
    if trace:
        return full, res
    return full


# revision 30
# speedup vs baseline: 1.1720x; 1.1720x over previous
"""CharEmb kernel for Trainium2 (8 NeuronCores, batch-sharded).

Computation (per word of 32 chars):
  emb = table[ids]                  # [32 chars, 64] per word
  x[i, j] = emb[i//2, 32*(i%2)+j]   # raw-buffer reshape [64, 32]
  y[f, t] = sum_{i,k} x[i, t+k] * w[f, i, k]   (valid conv, K=3)
  out[f] = max_t y[f, t] + b[f]

Data-parallel: char_ids batch-sharded 2048 words/core; table + conv
weights replicated (inference only, no grads). Pipeline per chunk pair
(64 words, 2048 chars), deeply buffered so the PE stays busy and the
HAM clock-gate spends most of the kernel at 2.4 GHz:
  1. DMA: host-precomputed one-hot [128(pad), 2048] bf16, permuted
     column order (within each 512-col half, col 64*(w//2)+2p+(w%2)
     holds char (w, p)); 128 partitions so the descriptors spread
     round-robin over all 16 SDMA engines (101 rows would serialize
     onto one engine at 27 GB/s).
  2. PE gather, column-group tiled: two M=64 matmuls per chunk at
     tile_position (0,0)/(0,64) run concurrently in disjoint column
     groups of the 128x128 array -> PSUM [128, 512] (char halves
     stacked on partitions).
  3. Act: PSUM -> SBUF bf16 copy [128, 512] per chunk.
  4. DVE: packed 32x32 stream transposes (int32 views of bf16 pairs):
     t_a rows 0:63 = A-data (words 0:15), t_b rows 64:127 = B-data
     (words 16:31, partition-swapped so the tap-2 matmuls can
     row-tile); T[32h+p, 64u+2j+eps] = emb_{w=2u+eps}[p, 32h+j].
  5. DMA shift-dups: the other 64 rows of each tile = data shifted
     +2 bf16 cols (tap k=1 operand).
  6. PE conv per chunk: fused taps 0,1 (128-contraction, stationary
     [W0;W1] for A, [W1;W0] for swapped B) + tap 2 row-tiled: A on
     array rows 0:63 (tile_position (0,0)), B on rows 64:127 ((64,0));
     the two 480-col tap-2 streams overlap in the array.
  7. maxpool over t=30 per chunk: Act compacts the 960 used cols ->
     SBUF bf16; DVE 16-wide overlapping max (covers t0..29, double-
     counting t14/t15 harmlessly) then one 16->1 tensor_reduce.
  8. end: DVE per-partition bias add, one output DMA.
"""

import sys
from contextlib import ExitStack

import numpy as np

if "/opt/trn_rl_repo" not in sys.path:
    sys.path.insert(0, "/opt/trn_rl_repo")

import concourse.bass as bass
import concourse.tile as tile
from concourse import bacc, mybir
from concourse.bass_utils import run_bass_kernel_spmd

# Problem constants (hardcoded per spec)
B, S, C = 32, 512, 32
V, E = 101, 64
F, K = 128, 3
T = C - K + 1  # 30 valid conv positions
NCORES = 8
WORDS = (B * S) // NCORES  # 2048 words per core
NCHARS = WORDS * C  # 65536

CHUNK_W = 32                 # words per chunk
CH_COLS = CHUNK_W * C        # 1024 chars per chunk
NCHUNKS = WORDS // CHUNK_W   # 64
HALF_W = 16                  # words per T-tile half
HALF_COLS = HALF_W * C       # 512

f32 = mybir.dt.float32
bf16 = mybir.dt.bfloat16
i32 = mybir.dt.int32

# maxpool strategy per chunk (cycled): 'a' Act copy + DVE L1+reduce,
# 'd' direct DVE reduce from PSUM
POOL_PATTERN = "a"


def build_kernel(num_devices=NCORES):
    nc = bacc.Bacc(
        "TRN2",
        target_bir_lowering=False,
        debug=False,
        enable_asserts=True,
        num_devices=num_devices,
    )

    oh_d = nc.dram_tensor("oh", [128, NCHARS], bf16, kind="ExternalInput")
    tab_d = nc.dram_tensor("tab", [128, E], bf16, kind="ExternalInput")
    w_d = nc.dram_tensor("wmat", [128, 384], bf16, kind="ExternalInput")
    b_d = nc.dram_tensor("bias", [128, 1], f32, kind="ExternalInput")
    out_d = nc.dram_tensor("out", [128, WORDS], f32, kind="ExternalOutput")

    with tile.TileContext(nc) as tc, ExitStack() as ctx:
        const_pool = ctx.enter_context(tc.tile_pool(name="const", bufs=1))
        oh_pool = ctx.enter_context(tc.tile_pool(name="oh", bufs=8))
        gsb_pool = ctx.enter_context(tc.tile_pool(name="gsb", bufs=6))
        ta_pool = ctx.enter_context(tc.tile_pool(name="tta", bufs=4))
        tb_pool = ctx.enter_context(tc.tile_pool(name="ttb", bufs=4))
        ysb_pool = ctx.enter_context(tc.tile_pool(name="ysb", bufs=4))
        y2_pool = ctx.enter_context(tc.tile_pool(name="y2", bufs=4))
        ob_pool = ctx.enter_context(tc.tile_pool(name="ob", bufs=3))
        g_psum = ctx.enter_context(tc.tile_pool(name="gps", bufs=4, space="PSUM"))
        y_psum = ctx.enter_context(tc.tile_pool(name="yps", bufs=2, space="PSUM"))

        tab_sb = const_pool.tile([128, E], bf16)
        w_sb = const_pool.tile([128, 384], bf16)
        b_sb = const_pool.tile([128, 1], f32)

        nc.sync.dma_start(tab_sb[:], tab_d.ap())
        nc.sync.dma_start(w_sb[:], w_d.ap())
        nc.sync.dma_start(b_sb[:], b_d.ap())

        obt = None
        for pp in range(NCHUNKS // 2):  # chunk pairs
            if pp % 4 == 0:
                # fresh output tile per 8-chunk segment (256 words)
                obt = ob_pool.tile([128, 8 * CHUNK_W], f32)
            # one-hot for the pair, one DMA (128 partitions -> spread)
            oh_t = oh_pool.tile([128, 2 * CH_COLS], bf16)
            nc.sync.dma_start(
                oh_t[:, :],
                oh_d.ap()[:, pp * 2 * CH_COLS:(pp + 1) * 2 * CH_COLS],
            )

            t_a = ta_pool.tile([128, 2 * HALF_COLS], bf16)
            t_b = tb_pool.tile([128, 2 * HALF_COLS], bf16)

            g_list = []
            for cp in range(2):
                # gather matmuls, col-group tiled (concurrent pair)
                g_ps = g_psum.tile([128, HALF_COLS], f32)
                for hh in range(2):
                    nc.tensor.matmul(
                        g_ps[64 * hh:64 * (hh + 1), :],
                        tab_sb[0:128, 0:E],
                        oh_t[0:128, cp * CH_COLS + 512 * hh:
                             cp * CH_COLS + 512 * (hh + 1)],
                        start=True,
                        stop=True,
                        tile_position=(0, 64 * hh),
                    )
                g_list.append(g_ps)

            for cp in range(2):
                g_ps = g_list[cp]
                # PSUM -> SBUF bf16
                gsb = gsb_pool.tile([128, HALF_COLS], bf16)
                nc.scalar.copy(gsb[:, :], g_ps[:, :])
                # packed transposes: A-data -> t_a rows 0:63,
                # B-data -> t_b rows 64:127 (B-swapped layout)
                nc.vector.transpose(
                    t_a[0:64, 512 * cp:512 * (cp + 1)].bitcast(i32),
                    gsb[0:64, :].bitcast(i32),
                )
                nc.vector.transpose(
                    t_b[64:128, 512 * cp:512 * (cp + 1)].bitcast(i32),
                    gsb[64:128, :].bitcast(i32),
                )

            # shift-dups (+2 bf16 cols = +1 j)
            nc.sync.dma_start(
                t_a[64:128, :].rearrange("q (r x) -> q r x", r=2)
                [:, :, 0:510],
                t_a[0:64, :].rearrange("q (r x) -> q r x", r=2)
                [:, :, 2:512],
            )
            nc.sync.dma_start(
                t_b[0:64, :].rearrange("q (r x) -> q r x", r=2)
                [:, :, 0:510],
                t_b[64:128, :].rearrange("q (r x) -> q r x", r=2)
                [:, :, 2:512],
            )

            # conv; taps01 (full array) then row-tiled tap2 A/B pairs
            y_list = []
            v01 = []
            v2 = []
            for cp in range(2):
                y_ps = y_psum.tile([128, 2 * 512], f32)
                y_list.append(y_ps)
                for r, t_t in enumerate((t_a, t_b)):
                    tf = (
                        t_t[:, 512 * cp:512 * (cp + 1)]
                        .rearrange("q (u j e) -> q u e j", j=C, e=2)
                    )
                    if r == 0:
                        tl = (
                            t_a[0:64, 512 * cp:512 * (cp + 1)]
                            .rearrange("q (u j e) -> q u e j", j=C, e=2)
                        )
                        w2 = w_sb[0:64, 128:256]
                        tpos = (0, 0)
                    else:
                        tl = (
                            t_b[64:128, 512 * cp:512 * (cp + 1)]
                            .rearrange("q (u j e) -> q u e j", j=C, e=2)
                        )
                        w2 = w_sb[64:128, 128:256]
                        tpos = (64, 0)
                    out_ap = (
                        y_ps[:, 512 * r:512 * r + HALF_W * T]
                        .rearrange("f (u e t) -> f u e t", t=T, e=2)
                    )
                    w01 = w_sb[:, 0:128] if r == 0 else w_sb[:, 256:384]
                    v01.append((out_ap, w01, tf))
                    v2.append((out_ap, w2, tl, tpos))
            for (out_ap, w01, tf) in v01:
                nc.tensor.matmul(
                    out_ap, w01, tf[:, :, :, 0:T],
                    start=True, stop=False, skip_group_check=True,
                )
            for (out_ap, w2, tl, tpos) in v2:
                nc.tensor.matmul(
                    out_ap, w2, tl[:, :, :, 2:2 + T],
                    start=False, stop=True, skip_group_check=True,
                    tile_position=tpos,
                )

            # maxpool over t -> obuf
            for cp in range(2):
                cc = 2 * pp + cp
                y_ps = y_list[cp]
                strat = POOL_PATTERN[cc % len(POOL_PATTERN)]
                if strat == "d":
                    # direct DVE reduce from PSUM
                    red_in = (
                        y_ps[:, :].rearrange("f (r x) -> f r x", x=512)
                        [:, :, 0:HALF_W * T]
                        .rearrange("f r (w t) -> f r w t", t=T)
                    )
                    nc.vector.tensor_reduce(
                        obt[:, (cc % 8) * CHUNK_W:(cc % 8 + 1) * CHUNK_W]
                        .rearrange("f (r w) -> f r w", w=HALF_W),
                        red_in, axis=mybir.AxisListType.X,
                        op=mybir.AluOpType.max,
                    )
                else:
                    # Act compacts used cols -> SBUF bf16 [128, 960]
                    ysb = ysb_pool.tile([128, 2 * HALF_W * T], bf16)
                    nc.scalar.copy(
                        ysb[:, :].rearrange("f (r y) -> f r y", r=2),
                        y_ps[:, :].rearrange("f (r x) -> f r x", x=512)
                        [:, :, 0:HALF_W * T],
                    )
                    yv = ysb[:, :].rearrange("f (g t) -> f g t", t=T)
                    y2 = y2_pool.tile([128, 32 * 16], bf16)
                    y2v = y2[:, :].rearrange("f (g t) -> f g t", t=16)
                    # L1: 16-wide overlapping max covers t 0..29
                    nc.vector.tensor_tensor(
                        y2v[:, :, :],
                        yv[:, :, 0:16],
                        yv[:, :, 14:30],
                        op=mybir.AluOpType.max,
                    )
                    # L2: 16 -> 8 (2x-rate tensor_tensor)
                    nc.vector.tensor_tensor(
                        y2v[:, :, 0:8],
                        y2v[:, :, 0:8],
                        y2v[:, :, 8:16],
                        op=mybir.AluOpType.max,
                    )
                    # L3: single 8 -> 1 reduce (1x rate, but half the cols)
                    nc.vector.tensor_reduce(
                        obt[:, (cc % 8) * CHUNK_W:(cc % 8 + 1) * CHUNK_W],
                        y2v[:, :, 0:8], axis=mybir.AxisListType.X,
                        op=mybir.AluOpType.max,
                    )
                if cc % 8 == 7:
                    # segment done: bias-add on Act and store, overlapped
                    # with later pairs' compute
                    seg = slice((cc - 7) * CHUNK_W, (cc + 1) * CHUNK_W)
                    nc.vector.tensor_scalar_add(
                        obt[:, :], obt[:, :], b_sb[:, 0:1])
                    nc.gpsimd.dma_start(out_d.ap()[:, seg], obt[:, :])



    nc.compile()
    return nc


def host_prep(char_ids, emb_table, conv_w, conv_b, num_devices=NCORES):
    """Build per-core input maps from full inputs."""
    char_ids = np.asarray(char_ids)
    emb_table = np.asarray(emb_table, dtype=np.float32)
    conv_w = np.asarray(conv_w, dtype=np.float32)
    conv_b = np.asarray(conv_b, dtype=np.float32)

    bf = mybir.dt.np(bf16)
    tab = np.zeros((128, E), dtype=np.float32)
    tab[0:V] = emb_table
    tab = tab.astype(bf)

    # permuted ids: within each 512-col half (16 words), column
    # 64*(w'//2) + 2p + (w'%2) holds char (w', p)
    ids_all = char_ids.reshape(-1, HALF_W, C).astype(np.int64)  # [nh, w', p]
    nh = ids_all.shape[0]
    ids_perm = (
        ids_all.reshape(nh, HALF_W // 2, 2, C)
        .transpose(0, 1, 3, 2)  # [half, u, p, eps]
        .reshape(nh * HALF_COLS)
    ).astype(np.int32)

    # one-hot of the permuted ids, zero-padded to 128 rows so the
    # DRAM->SBUF DMA spreads across all 16 SDMA engines
    oh_all = (ids_perm[None, :] == np.arange(128, dtype=np.int32)[:, None])
    oh_all = oh_all.astype(bf)

    # W layout: q = 32h+p (q<64) -> channel 2p+h
    q = np.arange(64)
    ch = 2 * (q % 32) + q // 32
    wmat = np.zeros((128, 384), dtype=np.float32)
    # A stationary [W0; W1]
    wmat[0:64, 0:128] = conv_w[:, ch, 0].T
    wmat[64:128, 0:128] = conv_w[:, ch, 1].T
    # tap2 for A (rows 0:63) and B (rows 64:127)
    wmat[0:64, 128:256] = conv_w[:, ch, 2].T
    wmat[64:128, 128:256] = conv_w[:, ch, 2].T
    # B stationary [W1; W0] (B tile is partition-swapped: dup low, data high)
    wmat[0:64, 256:384] = conv_w[:, ch, 1].T
    wmat[64:128, 256:384] = conv_w[:, ch, 0].T
    wmat = wmat.astype(bf)

    bias = conv_b.reshape(128, 1).astype(np.float32)

    in_maps = []
    for jcore in range(num_devices):
        oh_core = oh_all[:, jcore * NCHARS:(jcore + 1) * NCHARS]
        in_maps.append(
            {
                "oh": np.ascontiguousarray(oh_core),
                "tab": tab,
                "wmat": wmat,
                "bias": bias,
            }
        )
    return in_maps


def _ensure_ntff_hook():
    """The agent image's antenv lacks axon_hooks; shim it and install the
    ctypes NTFF profiling hook so trace=True yields HW exec times."""
    import types

    if "antenv.axon_hooks" in sys.modules:
        return
    mod = types.ModuleType("antenv.axon_hooks")
    _hook = [None]
    mod.get_axon_ntff_profile_hook = lambda: _hook[0]
    mod.set_axon_ntff_profile_hook = lambda h: _hook.__setitem__(0, h)
    sys.modules["antenv.axon_hooks"] = mod
    try:
        import antenv

        antenv.axon_hooks = mod
        from trn_agent_boot.trn_boot import _ntff_profile_via_ctypes

        hook = _ntff_profile_via_ctypes("/opt/axon/libaxon_pjrt.so")
        mod.set_axon_ntff_profile_hook(hook)
    except Exception as e:  # degrade to no-trace
        print(f"ntff hook install failed: {e}", file=sys.stderr)


_NC_CACHE = {}


def _get_nc():
    if "nc" not in _NC_CACHE:
        _NC_CACHE["nc"] = build_kernel()
    return _NC_CACHE["nc"]


def kernel(char_ids, emb_table, conv_w, conv_b, trace=False):
    if trace:
        _ensure_ntff_hook()
    nc = _get_nc()
    in_maps = host_prep(char_ids, emb_table, conv_w, conv_b)
    res = run_bass_kernel_spmd(
        nc, in_maps, core_ids=list(range(NCORES)), trace=trace
    )
    # out[f, word] word-linear -> [word, f]
    outs = [res.results[jc]["out"].T for jc in range(NCORES)]
    full = np.concatenate(outs, axis=0).reshape(B, S, F).astype(np.float32)
    if trace:
        return full, res
    return full


# revision 31
# speedup vs baseline: 1.1888x; 1.0143x over previous
"""CharEmb kernel for Trainium2 (8 NeuronCores, batch-sharded).

Computation (per word of 32 chars):
  emb = table[ids]                  # [32 chars, 64] per word
  x[i, j] = emb[i//2, 32*(i%2)+j]   # raw-buffer reshape [64, 32]
  y[f, t] = sum_{i,k} x[i, t+k] * w[f, i, k]   (valid conv, K=3)
  out[f] = max_t y[f, t] + b[f]

Data-parallel: char_ids batch-sharded 2048 words/core; table + conv
weights replicated (inference only, no grads). Pipeline per chunk pair
(64 words, 2048 chars), deeply buffered so the PE stays busy and the
HAM clock-gate spends most of the kernel at 2.4 GHz:
  1. DMA: host-precomputed one-hot [128(pad), 2048] bf16, permuted
     column order (within each 512-col half, col 64*(w//2)+2p+(w%2)
     holds char (w, p)); 128 partitions so the descriptors spread
     round-robin over all 16 SDMA engines (101 rows would serialize
     onto one engine at 27 GB/s).
  2. PE gather, column-group tiled: two M=64 matmuls per chunk at
     tile_position (0,0)/(0,64) run concurrently in disjoint column
     groups of the 128x128 array -> PSUM [128, 512] (char halves
     stacked on partitions).
  3. Act: PSUM -> SBUF bf16 copy [128, 512] per chunk.
  4. DVE: packed 32x32 stream transposes (int32 views of bf16 pairs):
     t_a rows 0:63 = A-data (words 0:15), t_b rows 64:127 = B-data
     (words 16:31, partition-swapped so the tap-2 matmuls can
     row-tile); T[32h+p, 64u+2j+eps] = emb_{w=2u+eps}[p, 32h+j].
  5. DMA shift-dups: the other 64 rows of each tile = data shifted
     +2 bf16 cols (tap k=1 operand).
  6. PE conv per chunk: fused taps 0,1 (128-contraction, stationary
     [W0;W1] for A, [W1;W0] for swapped B) + tap 2 row-tiled: A on
     array rows 0:63 (tile_position (0,0)), B on rows 64:127 ((64,0));
     the two 480-col tap-2 streams overlap in the array.
  7. maxpool over t=30 per chunk: Act compacts the 960 used cols ->
     SBUF bf16; DVE 16-wide overlapping max (covers t0..29, double-
     counting t14/t15 harmlessly) then one 16->1 tensor_reduce.
  8. end: DVE per-partition bias add, one output DMA.
"""

import sys
from contextlib import ExitStack

import numpy as np

if "/opt/trn_rl_repo" not in sys.path:
    sys.path.insert(0, "/opt/trn_rl_repo")

import concourse.bass as bass
import concourse.tile as tile
from concourse import bacc, mybir
from concourse.bass_utils import run_bass_kernel_spmd

# Problem constants (hardcoded per spec)
B, S, C = 32, 512, 32
V, E = 101, 64
F, K = 128, 3
T = C - K + 1  # 30 valid conv positions
NCORES = 8
WORDS = (B * S) // NCORES  # 2048 words per core
NCHARS = WORDS * C  # 65536

CHUNK_W = 32                 # words per chunk
CH_COLS = CHUNK_W * C        # 1024 chars per chunk
NCHUNKS = WORDS // CHUNK_W   # 64
HALF_W = 16                  # words per T-tile half
HALF_COLS = HALF_W * C       # 512

f32 = mybir.dt.float32
bf16 = mybir.dt.bfloat16
i32 = mybir.dt.int32

# maxpool strategy per chunk (cycled): 'a' Act copy + DVE L1+reduce,
# 'd' direct DVE reduce from PSUM
POOL_PATTERN = "a"


def build_kernel(num_devices=NCORES):
    nc = bacc.Bacc(
        "TRN2",
        target_bir_lowering=False,
        debug=False,
        enable_asserts=True,
        num_devices=num_devices,
    )

    oh_d = nc.dram_tensor("oh", [128, NCHARS], bf16, kind="ExternalInput")
    tab_d = nc.dram_tensor("tab", [128, E], bf16, kind="ExternalInput")
    w_d = nc.dram_tensor("wmat", [128, 384], bf16, kind="ExternalInput")
    b_d = nc.dram_tensor("bias", [128, 1], f32, kind="ExternalInput")
    out_d = nc.dram_tensor("out", [128, WORDS], f32, kind="ExternalOutput")

    with tile.TileContext(nc) as tc, ExitStack() as ctx:
        const_pool = ctx.enter_context(tc.tile_pool(name="const", bufs=1))
        oh_pool = ctx.enter_context(tc.tile_pool(name="oh", bufs=8))
        gsb_pool = ctx.enter_context(tc.tile_pool(name="gsb", bufs=6))
        ta_pool = ctx.enter_context(tc.tile_pool(name="tta", bufs=4))
        tb_pool = ctx.enter_context(tc.tile_pool(name="ttb", bufs=4))
        ysb_pool = ctx.enter_context(tc.tile_pool(name="ysb", bufs=4))
        y2_pool = ctx.enter_context(tc.tile_pool(name="y2", bufs=4))
        ob_pool = ctx.enter_context(tc.tile_pool(name="ob", bufs=3))
        g_psum = ctx.enter_context(tc.tile_pool(name="gps", bufs=4, space="PSUM"))
        y_psum = ctx.enter_context(tc.tile_pool(name="yps", bufs=2, space="PSUM"))

        tab_sb = const_pool.tile([128, E], bf16)
        w_sb = const_pool.tile([128, 384], bf16)
        b_sb = const_pool.tile([128, 1], f32)

        nc.sync.dma_start(tab_sb[:], tab_d.ap())
        nc.sync.dma_start(w_sb[:], w_d.ap())
        nc.sync.dma_start(b_sb[:], b_d.ap())

        obt = None
        for pp in range(NCHUNKS // 2):  # chunk pairs
            if pp % 2 == 0:
                # fresh output tile per 4-chunk segment (128 words)
                obt = ob_pool.tile([128, 4 * CHUNK_W], f32)
            # one-hot for the pair, one DMA (128 partitions -> spread)
            oh_t = oh_pool.tile([128, 2 * CH_COLS], bf16)
            nc.sync.dma_start(
                oh_t[:, :],
                oh_d.ap()[:, pp * 2 * CH_COLS:(pp + 1) * 2 * CH_COLS],
            )

            t_a = ta_pool.tile([128, 2 * HALF_COLS], bf16)
            t_b = tb_pool.tile([128, 2 * HALF_COLS], bf16)

            g_list = []
            for cp in range(2):
                # gather matmuls, col-group tiled (concurrent pair)
                g_ps = g_psum.tile([128, HALF_COLS], f32)
                for hh in range(2):
                    nc.tensor.matmul(
                        g_ps[64 * hh:64 * (hh + 1), :],
                        tab_sb[0:128, 0:E],
                        oh_t[0:128, cp * CH_COLS + 512 * hh:
                             cp * CH_COLS + 512 * (hh + 1)],
                        start=True,
                        stop=True,
                        tile_position=(0, 64 * hh),
                    )
                g_list.append(g_ps)

            for cp in range(2):
                g_ps = g_list[cp]
                # PSUM -> SBUF bf16
                gsb = gsb_pool.tile([128, HALF_COLS], bf16)
                nc.scalar.copy(gsb[:, :], g_ps[:, :])
                # packed transposes: A-data -> t_a rows 0:63,
                # B-data -> t_b rows 64:127 (B-swapped layout)
                nc.vector.transpose(
                    t_a[0:64, 512 * cp:512 * (cp + 1)].bitcast(i32),
                    gsb[0:64, :].bitcast(i32),
                )
                nc.vector.transpose(
                    t_b[64:128, 512 * cp:512 * (cp + 1)].bitcast(i32),
                    gsb[64:128, :].bitcast(i32),
                )

            # shift-dups (+2 bf16 cols = +1 j)
            nc.sync.dma_start(
                t_a[64:128, :].rearrange("q (r x) -> q r x", r=2)
                [:, :, 0:510],
                t_a[0:64, :].rearrange("q (r x) -> q r x", r=2)
                [:, :, 2:512],
            )
            nc.sync.dma_start(
                t_b[0:64, :].rearrange("q (r x) -> q r x", r=2)
                [:, :, 0:510],
                t_b[64:128, :].rearrange("q (r x) -> q r x", r=2)
                [:, :, 2:512],
            )

            # conv; taps01 (full array) then row-tiled tap2 A/B pairs
            y_list = []
            v01 = []
            v2 = []
            for cp in range(2):
                y_ps = y_psum.tile([128, 2 * 512], f32)
                y_list.append(y_ps)
                for r, t_t in enumerate((t_a, t_b)):
                    tf = (
                        t_t[:, 512 * cp:512 * (cp + 1)]
                        .rearrange("q (u j e) -> q u e j", j=C, e=2)
                    )
                    if r == 0:
                        tl = (
                            t_a[0:64, 512 * cp:512 * (cp + 1)]
                            .rearrange("q (u j e) -> q u e j", j=C, e=2)
                        )
                        w2 = w_sb[0:64, 128:256]
                        tpos = (0, 0)
                    else:
                        tl = (
                            t_b[64:128, 512 * cp:512 * (cp + 1)]
                            .rearrange("q (u j e) -> q u e j", j=C, e=2)
                        )
                        w2 = w_sb[64:128, 128:256]
                        tpos = (64, 0)
                    out_ap = (
                        y_ps[:, 512 * r:512 * r + HALF_W * T]
                        .rearrange("f (u e t) -> f u e t", t=T, e=2)
                    )
                    w01 = w_sb[:, 0:128] if r == 0 else w_sb[:, 256:384]
                    v01.append((out_ap, w01, tf))
                    v2.append((out_ap, w2, tl, tpos))
            for (out_ap, w01, tf) in v01:
                nc.tensor.matmul(
                    out_ap, w01, tf[:, :, :, 0:T],
                    start=True, stop=False, skip_group_check=True,
                )
            for (out_ap, w2, tl, tpos) in v2:
                nc.tensor.matmul(
                    out_ap, w2, tl[:, :, :, 2:2 + T],
                    start=False, stop=True, skip_group_check=True,
                    tile_position=tpos,
                )

            # maxpool over t -> obuf
            for cp in range(2):
                cc = 2 * pp + cp
                y_ps = y_list[cp]
                strat = POOL_PATTERN[cc % len(POOL_PATTERN)]
                if strat == "d":
                    # direct DVE reduce from PSUM
                    red_in = (
                        y_ps[:, :].rearrange("f (r x) -> f r x", x=512)
                        [:, :, 0:HALF_W * T]
                        .rearrange("f r (w t) -> f r w t", t=T)
                    )
                    nc.vector.tensor_reduce(
                        obt[:, (cc % 4) * CHUNK_W:(cc % 4 + 1) * CHUNK_W]
                        .rearrange("f (r w) -> f r w", w=HALF_W),
                        red_in, axis=mybir.AxisListType.X,
                        op=mybir.AluOpType.max,
                    )
                else:
                    # Act compacts used cols -> SBUF bf16 [128, 960]
                    ysb = ysb_pool.tile([128, 2 * HALF_W * T], bf16)
                    nc.scalar.copy(
                        ysb[:, :].rearrange("f (r y) -> f r y", r=2),
                        y_ps[:, :].rearrange("f (r x) -> f r x", x=512)
                        [:, :, 0:HALF_W * T],
                    )
                    yv = ysb[:, :].rearrange("f (g t) -> f g t", t=T)
                    y2 = y2_pool.tile([128, 32 * 16], bf16)
                    y2v = y2[:, :].rearrange("f (g t) -> f g t", t=16)
                    # L1: 16-wide overlapping max covers t 0..29
                    nc.vector.tensor_tensor(
                        y2v[:, :, :],
                        yv[:, :, 0:16],
                        yv[:, :, 14:30],
                        op=mybir.AluOpType.max,
                    )
                    # L2: 16 -> 8 (2x-rate tensor_tensor)
                    nc.vector.tensor_tensor(
                        y2v[:, :, 0:8],
                        y2v[:, :, 0:8],
                        y2v[:, :, 8:16],
                        op=mybir.AluOpType.max,
                    )
                    # L3: single 8 -> 1 reduce (1x rate, but half the cols)
                    nc.vector.tensor_reduce(
                        obt[:, (cc % 4) * CHUNK_W:(cc % 4 + 1) * CHUNK_W],
                        y2v[:, :, 0:8], axis=mybir.AxisListType.X,
                        op=mybir.AluOpType.max,
                    )
                if cc % 4 == 3:
                    # segment done: bias-add + store, overlapped with
                    # later pairs' compute
                    seg = slice((cc - 3) * CHUNK_W, (cc + 1) * CHUNK_W)
                    nc.vector.tensor_scalar_add(
                        obt[:, :], obt[:, :], b_sb[:, 0:1])
                    nc.gpsimd.dma_start(out_d.ap()[:, seg], obt[:, :])



    nc.compile()
    return nc


def host_prep(char_ids, emb_table, conv_w, conv_b, num_devices=NCORES):
    """Build per-core input maps from full inputs."""
    char_ids = np.asarray(char_ids)
    emb_table = np.asarray(emb_table, dtype=np.float32)
    conv_w = np.asarray(conv_w, dtype=np.float32)
    conv_b = np.asarray(conv_b, dtype=np.float32)

    bf = mybir.dt.np(bf16)
    tab = np.zeros((128, E), dtype=np.float32)
    tab[0:V] = emb_table
    tab = tab.astype(bf)

    # permuted ids: within each 512-col half (16 words), column
    # 64*(w'//2) + 2p + (w'%2) holds char (w', p)
    ids_all = char_ids.reshape(-1, HALF_W, C).astype(np.int64)  # [nh, w', p]
    nh = ids_all.shape[0]
    ids_perm = (
        ids_all.reshape(nh, HALF_W // 2, 2, C)
        .transpose(0, 1, 3, 2)  # [half, u, p, eps]
        .reshape(nh * HALF_COLS)
    ).astype(np.int32)

    # one-hot of the permuted ids, zero-padded to 128 rows so the
    # DRAM->SBUF DMA spreads across all 16 SDMA engines
    oh_all = (ids_perm[None, :] == np.arange(128, dtype=np.int32)[:, None])
    oh_all = oh_all.astype(bf)

    # W layout: q = 32h+p (q<64) -> channel 2p+h
    q = np.arange(64)
    ch = 2 * (q % 32) + q // 32
    wmat = np.zeros((128, 384), dtype=np.float32)
    # A stationary [W0; W1]
    wmat[0:64, 0:128] = conv_w[:, ch, 0].T
    wmat[64:128, 0:128] = conv_w[:, ch, 1].T
    # tap2 for A (rows 0:63) and B (rows 64:127)
    wmat[0:64, 128:256] = conv_w[:, ch, 2].T
    wmat[64:128, 128:256] = conv_w[:, ch, 2].T
    # B stationary [W1; W0] (B tile is partition-swapped: dup low, data high)
    wmat[0:64, 256:384] = conv_w[:, ch, 1].T
    wmat[64:128, 256:384] = conv_w[:, ch, 0].T
    wmat = wmat.astype(bf)

    bias = conv_b.reshape(128, 1).astype(np.float32)

    in_maps = []
    for jcore in range(num_devices):
        oh_core = oh_all[:, jcore * NCHARS:(jcore + 1) * NCHARS]
        in_maps.append(
            {
                "oh": np.ascontiguousarray(oh_core),
                "tab": tab,
                "wmat": wmat,
                "bias": bias,
            }
        )
    return in_maps


def _ensure_ntff_hook():
    """The agent image's antenv lacks axon_hooks; shim it and install the
    ctypes NTFF profiling hook so trace=True yields HW exec times."""
    import types

    if "antenv.axon_hooks" in sys.modules:
        return
    mod = types.ModuleType("antenv.axon_hooks")
    _hook = [None]
    mod.get_axon_ntff_profile_hook = lambda: _hook[0]
    mod.set_axon_ntff_profile_hook = lambda h: _hook.__setitem__(0, h)
    sys.modules["antenv.axon_hooks"] = mod
    try:
        import antenv

        antenv.axon_hooks = mod
        from trn_agent_boot.trn_boot import _ntff_profile_via_ctypes

        hook = _ntff_profile_via_ctypes("/opt/axon/libaxon_pjrt.so")
        mod.set_axon_ntff_profile_hook(hook)
    except Exception as e:  # degrade to no-trace
        print(f"ntff hook install failed: {e}", file=sys.stderr)


_NC_CACHE = {}


def _get_nc():
    if "nc" not in _NC_CACHE:
        _NC_CACHE["nc"] = build_kernel()
    return _NC_CACHE["nc"]


def kernel(char_ids, emb_table, conv_w, conv_b, trace=False):
    if trace:
        _ensure_ntff_hook()
    nc = _get_nc()
    in_maps = host_prep(char_ids, emb_table, conv_w, conv_b)
    res = run_bass_kernel_spmd(
        nc, in_maps, core_ids=list(range(NCORES)), trace=trace
    )
    # out[f, word] word-linear -> [word, f]
    outs = [res.results[jc]["out"].T for jc in range(NCORES)]
    full = np.concatenate(outs, axis=0).reshape(B, S, F).astype(np.float32)
    if trace:
        return full, res
    return full


# revision 32
# speedup vs baseline: 1.2020x; 1.0111x over previous
"""CharEmb kernel for Trainium2 (8 NeuronCores, batch-sharded).

Computation (per word of 32 chars):
  emb = table[ids]                  # [32 chars, 64] per word
  x[i, j] = emb[i//2, 32*(i%2)+j]   # raw-buffer reshape [64, 32]
  y[f, t] = sum_{i,k} x[i, t+k] * w[f, i, k]   (valid conv, K=3)
  out[f] = max_t y[f, t] + b[f]

Data-parallel: char_ids batch-sharded 2048 words/core; table + conv
weights replicated (inference only, no grads). Pipeline per chunk pair
(64 words, 2048 chars), deeply buffered so the PE stays busy and the
HAM clock-gate spends most of the kernel at 2.4 GHz:
  1. DMA: host-precomputed one-hot [128(pad), 2048] bf16, permuted
     column order (within each 512-col half, col 64*(w//2)+2p+(w%2)
     holds char (w, p)); 128 partitions so the descriptors spread
     round-robin over all 16 SDMA engines (101 rows would serialize
     onto one engine at 27 GB/s).
  2. PE gather, column-group tiled: two M=64 matmuls per chunk at
     tile_position (0,0)/(0,64) run concurrently in disjoint column
     groups of the 128x128 array -> PSUM [128, 512] (char halves
     stacked on partitions).
  3. Act: PSUM -> SBUF bf16 copy [128, 512] per chunk.
  4. DVE: packed 32x32 stream transposes (int32 views of bf16 pairs):
     t_a rows 0:63 = A-data (words 0:15), t_b rows 64:127 = B-data
     (words 16:31, partition-swapped so the tap-2 matmuls can
     row-tile); T[32h+p, 64u+2j+eps] = emb_{w=2u+eps}[p, 32h+j].
  5. DMA shift-dups: the other 64 rows of each tile = data shifted
     +2 bf16 cols (tap k=1 operand).
  6. PE conv per chunk: fused taps 0,1 (128-contraction, stationary
     [W0;W1] for A, [W1;W0] for swapped B) + tap 2 row-tiled: A on
     array rows 0:63 (tile_position (0,0)), B on rows 64:127 ((64,0));
     the two 480-col tap-2 streams overlap in the array.
  7. maxpool over t=30 per chunk: Act compacts the 960 used cols ->
     SBUF bf16; DVE 16-wide overlapping max (covers t0..29, double-
     counting t14/t15 harmlessly), a 16->8 max, then an 8->1 reduce
     (tensor_tensor runs 2x rate, tensor_reduce only 1x, so trimming
     the reduce width wins).
  8. outputs in 4-chunk segment tiles: DVE bias add (in-queue after its
     own producer, no FIFO head-of-line stall) + store via the idle
     gpsimd SWDGE queue, overlapped with later pairs' compute.
"""

import sys
from contextlib import ExitStack

import numpy as np

if "/opt/trn_rl_repo" not in sys.path:
    sys.path.insert(0, "/opt/trn_rl_repo")

import concourse.bass as bass
import concourse.tile as tile
from concourse import bacc, mybir
from concourse.bass_utils import run_bass_kernel_spmd

# Problem constants (hardcoded per spec)
B, S, C = 32, 512, 32
V, E = 101, 64
F, K = 128, 3
T = C - K + 1  # 30 valid conv positions
NCORES = 8
WORDS = (B * S) // NCORES  # 2048 words per core
NCHARS = WORDS * C  # 65536

CHUNK_W = 32                 # words per chunk
CH_COLS = CHUNK_W * C        # 1024 chars per chunk
NCHUNKS = WORDS // CHUNK_W   # 64
HALF_W = 16                  # words per T-tile half
HALF_COLS = HALF_W * C       # 512

f32 = mybir.dt.float32
bf16 = mybir.dt.bfloat16
i32 = mybir.dt.int32

# maxpool strategy per chunk (cycled): 'a' Act copy + DVE L1+reduce,
# 'd' direct DVE reduce from PSUM
POOL_PATTERN = "a"


def build_kernel(num_devices=NCORES):
    nc = bacc.Bacc(
        "TRN2",
        target_bir_lowering=False,
        debug=False,
        enable_asserts=True,
        num_devices=num_devices,
    )

    oh_d = nc.dram_tensor("oh", [128, NCHARS], bf16, kind="ExternalInput")
    tab_d = nc.dram_tensor("tab", [128, E], bf16, kind="ExternalInput")
    w_d = nc.dram_tensor("wmat", [128, 384], bf16, kind="ExternalInput")
    b_d = nc.dram_tensor("bias", [128, 1], f32, kind="ExternalInput")
    out_d = nc.dram_tensor("out", [128, WORDS], f32, kind="ExternalOutput")

    with tile.TileContext(nc) as tc, ExitStack() as ctx:
        const_pool = ctx.enter_context(tc.tile_pool(name="const", bufs=1))
        oh_pool = ctx.enter_context(tc.tile_pool(name="oh", bufs=8))
        gsb_pool = ctx.enter_context(tc.tile_pool(name="gsb", bufs=6))
        ta_pool = ctx.enter_context(tc.tile_pool(name="tta", bufs=4))
        tb_pool = ctx.enter_context(tc.tile_pool(name="ttb", bufs=4))
        ysb_pool = ctx.enter_context(tc.tile_pool(name="ysb", bufs=4))
        y2_pool = ctx.enter_context(tc.tile_pool(name="y2", bufs=4))
        ob_pool = ctx.enter_context(tc.tile_pool(name="ob", bufs=3))
        g_psum = ctx.enter_context(tc.tile_pool(name="gps", bufs=4, space="PSUM"))
        y_psum = ctx.enter_context(tc.tile_pool(name="yps", bufs=2, space="PSUM"))

        tab_sb = const_pool.tile([128, E], bf16)
        w_sb = const_pool.tile([128, 384], bf16)
        b_sb = const_pool.tile([128, 1], f32)

        nc.sync.dma_start(tab_sb[:], tab_d.ap())
        nc.sync.dma_start(w_sb[:], w_d.ap())
        nc.sync.dma_start(b_sb[:], b_d.ap())

        obt = None
        for pp in range(NCHUNKS // 2):  # chunk pairs
            if pp % 2 == 0:
                # fresh output tile per 4-chunk segment (128 words)
                obt = ob_pool.tile([128, 4 * CHUNK_W], f32)
            # one-hot for the pair, one DMA (128 partitions -> spread)
            oh_t = oh_pool.tile([128, 2 * CH_COLS], bf16)
            nc.sync.dma_start(
                oh_t[:, :],
                oh_d.ap()[:, pp * 2 * CH_COLS:(pp + 1) * 2 * CH_COLS],
            )

            t_a = ta_pool.tile([128, 2 * HALF_COLS], bf16)
            t_b = tb_pool.tile([128, 2 * HALF_COLS], bf16)

            g_list = []
            for cp in range(2):
                # gather matmuls, col-group tiled (concurrent pair)
                g_ps = g_psum.tile([128, HALF_COLS], f32)
                for hh in range(2):
                    nc.tensor.matmul(
                        g_ps[64 * hh:64 * (hh + 1), :],
                        tab_sb[0:128, 0:E],
                        oh_t[0:128, cp * CH_COLS + 512 * hh:
                             cp * CH_COLS + 512 * (hh + 1)],
                        start=True,
                        stop=True,
                        tile_position=(0, 64 * hh),
                    )
                g_list.append(g_ps)

            for cp in range(2):
                g_ps = g_list[cp]
                # PSUM -> SBUF bf16
                gsb = gsb_pool.tile([128, HALF_COLS], bf16)
                nc.scalar.copy(gsb[:, :], g_ps[:, :])
                # packed transposes: A-data -> t_a rows 0:63,
                # B-data -> t_b rows 64:127 (B-swapped layout)
                nc.vector.transpose(
                    t_a[0:64, 512 * cp:512 * (cp + 1)].bitcast(i32),
                    gsb[0:64, :].bitcast(i32),
                )
                nc.vector.transpose(
                    t_b[64:128, 512 * cp:512 * (cp + 1)].bitcast(i32),
                    gsb[64:128, :].bitcast(i32),
                )

            # shift-dups (+2 bf16 cols = +1 j)
            nc.sync.dma_start(
                t_a[64:128, :].rearrange("q (r x) -> q r x", r=2)
                [:, :, 0:510],
                t_a[0:64, :].rearrange("q (r x) -> q r x", r=2)
                [:, :, 2:512],
            )
            nc.sync.dma_start(
                t_b[0:64, :].rearrange("q (r x) -> q r x", r=2)
                [:, :, 0:510],
                t_b[64:128, :].rearrange("q (r x) -> q r x", r=2)
                [:, :, 2:512],
            )

            # conv; taps01 (full array) then row-tiled tap2 A/B pairs
            y_list = []
            v01 = []
            v2 = []
            for cp in range(2):
                y_ps = y_psum.tile([128, 2 * 512], f32)
                y_list.append(y_ps)
                for r, t_t in enumerate((t_a, t_b)):
                    tf = (
                        t_t[:, 512 * cp:512 * (cp + 1)]
                        .rearrange("q (u j e) -> q u e j", j=C, e=2)
                    )
                    if r == 0:
                        tl = (
                            t_a[0:64, 512 * cp:512 * (cp + 1)]
                            .rearrange("q (u j e) -> q u e j", j=C, e=2)
                        )
                        w2 = w_sb[0:64, 128:256]
                        tpos = (0, 0)
                    else:
                        tl = (
                            t_b[64:128, 512 * cp:512 * (cp + 1)]
                            .rearrange("q (u j e) -> q u e j", j=C, e=2)
                        )
                        w2 = w_sb[64:128, 128:256]
                        tpos = (64, 0)
                    out_ap = (
                        y_ps[:, 512 * r:512 * r + HALF_W * T]
                        .rearrange("f (u e t) -> f u e t", t=T, e=2)
                    )
                    w01 = w_sb[:, 0:128] if r == 0 else w_sb[:, 256:384]
                    v01.append((out_ap, w01, tf))
                    v2.append((out_ap, w2, tl, tpos))
            for (out_ap, w01, tf) in v01:
                nc.tensor.matmul(
                    out_ap, w01, tf[:, :, :, 0:T],
                    start=True, stop=False, skip_group_check=True,
                )
            for (out_ap, w2, tl, tpos) in v2:
                nc.tensor.matmul(
                    out_ap, w2, tl[:, :, :, 2:2 + T],
                    start=False, stop=True, skip_group_check=True,
                    tile_position=tpos,
                )

            # maxpool over t -> obuf
            for cp in range(2):
                cc = 2 * pp + cp
                y_ps = y_list[cp]
                strat = POOL_PATTERN[cc % len(POOL_PATTERN)]
                if strat == "d":
                    # direct DVE reduce from PSUM
                    red_in = (
                        y_ps[:, :].rearrange("f (r x) -> f r x", x=512)
                        [:, :, 0:HALF_W * T]
                        .rearrange("f r (w t) -> f r w t", t=T)
                    )
                    nc.vector.tensor_reduce(
                        obt[:, (cc % 4) * CHUNK_W:(cc % 4 + 1) * CHUNK_W]
                        .rearrange("f (r w) -> f r w", w=HALF_W),
                        red_in, axis=mybir.AxisListType.X,
                        op=mybir.AluOpType.max,
                    )
                else:
                    # Act compacts used cols -> SBUF bf16 [128, 960]
                    ysb = ysb_pool.tile([128, 2 * HALF_W * T], bf16)
                    nc.scalar.copy(
                        ysb[:, :].rearrange("f (r y) -> f r y", r=2),
                        y_ps[:, :].rearrange("f (r x) -> f r x", x=512)
                        [:, :, 0:HALF_W * T],
                    )
                    yv = ysb[:, :].rearrange("f (g t) -> f g t", t=T)
                    y2 = y2_pool.tile([128, 32 * 16], bf16)
                    y2v = y2[:, :].rearrange("f (g t) -> f g t", t=16)
                    # L1: 16-wide overlapping max covers t 0..29
                    nc.vector.tensor_tensor(
                        y2v[:, :, :],
                        yv[:, :, 0:16],
                        yv[:, :, 14:30],
                        op=mybir.AluOpType.max,
                    )
                    # L2: 16 -> 8 (2x-rate tensor_tensor)
                    nc.vector.tensor_tensor(
                        y2v[:, :, 0:8],
                        y2v[:, :, 0:8],
                        y2v[:, :, 8:16],
                        op=mybir.AluOpType.max,
                    )
                    # L3: single 8 -> 1 reduce (1x rate, but half the cols)
                    nc.vector.tensor_reduce(
                        obt[:, (cc % 4) * CHUNK_W:(cc % 4 + 1) * CHUNK_W],
                        y2v[:, :, 0:8], axis=mybir.AxisListType.X,
                        op=mybir.AluOpType.max,
                    )
                if cc % 4 == 3:
                    # segment done: bias-add + store, overlapped with
                    # later pairs' compute
                    seg = slice((cc - 3) * CHUNK_W, (cc + 1) * CHUNK_W)
                    nc.vector.tensor_scalar_add(
                        obt[:, :], obt[:, :], b_sb[:, 0:1])
                    nc.gpsimd.dma_start(out_d.ap()[:, seg], obt[:, :])



    nc.compile()
    return nc


def host_prep(char_ids, emb_table, conv_w, conv_b, num_devices=NCORES):
    """Build per-core input maps from full inputs."""
    char_ids = np.asarray(char_ids)
    emb_table = np.asarray(emb_table, dtype=np.float32)
    conv_w = np.asarray(conv_w, dtype=np.float32)
    conv_b = np.asarray(conv_b, dtype=np.float32)

    bf = mybir.dt.np(bf16)
    tab = np.zeros((128, E), dtype=np.float32)
    tab[0:V] = emb_table
    tab = tab.astype(bf)

    # permuted ids: within each 512-col half (16 words), column
    # 64*(w'//2) + 2p + (w'%2) holds char (w', p)
    ids_all = char_ids.reshape(-1, HALF_W, C).astype(np.int64)  # [nh, w', p]
    nh = ids_all.shape[0]
    ids_perm = (
        ids_all.reshape(nh, HALF_W // 2, 2, C)
        .transpose(0, 1, 3, 2)  # [half, u, p, eps]
        .reshape(nh * HALF_COLS)
    ).astype(np.int32)

    # one-hot of the permuted ids, zero-padded to 128 rows so the
    # DRAM->SBUF DMA spreads across all 16 SDMA engines
    oh_all = (ids_perm[None, :] == np.arange(128, dtype=np.int32)[:, None])
    oh_all = oh_all.astype(bf)

    # W layout: q = 32h+p (q<64) -> channel 2p+h
    q = np.arange(64)
    ch = 2 * (q % 32) + q // 32
    wmat = np.zeros((128, 384), dtype=np.float32)
    # A stationary [W0; W1]
    wmat[0:64, 0:128] = conv_w[:, ch, 0].T
    wmat[64:128, 0:128] = conv_w[:, ch, 1].T
    # tap2 for A (rows 0:63) and B (rows 64:127)
    wmat[0:64, 128:256] = conv_w[:, ch, 2].T
    wmat[64:128, 128:256] = conv_w[:, ch, 2].T
    # B stationary [W1; W0] (B tile is partition-swapped: dup low, data high)
    wmat[0:64, 256:384] = conv_w[:, ch, 1].T
    wmat[64:128, 256:384] = conv_w[:, ch, 0].T
    wmat = wmat.astype(bf)

    bias = conv_b.reshape(128, 1).astype(np.float32)

    in_maps = []
    for jcore in range(num_devices):
        oh_core = oh_all[:, jcore * NCHARS:(jcore + 1) * NCHARS]
        in_maps.append(
            {
                "oh": np.ascontiguousarray(oh_core),
                "tab": tab,
                "wmat": wmat,
                "bias": bias,
            }
        )
    return in_maps


def _ensure_ntff_hook():
    """The agent image's antenv lacks axon_hooks; shim it and install the
    ctypes NTFF profiling hook so trace=True yields HW exec times."""
    import types

    if "antenv.axon_hooks" in sys.modules:
        return
    mod = types.ModuleType("antenv.axon_hooks")
    _hook = [None]
    mod.get_axon_ntff_profile_hook = lambda: _hook[0]
    mod.set_axon_ntff_profile_hook = lambda h: _hook.__setitem__(0, h)
    sys.modules["antenv.axon_hooks"] = mod
    try:
        import antenv

        antenv.axon_hooks = mod
        from trn_agent_boot.trn_boot import _ntff_profile_via_ctypes

        hook = _ntff_profile_via_ctypes("/opt/axon/libaxon_pjrt.so")
        mod.set_axon_ntff_profile_hook(hook)
    except Exception as e:  # degrade to no-trace
        print(f"ntff hook install failed: {e}", file=sys.stderr)


_NC_CACHE = {}


def _get_nc():
    if "nc" not in _NC_CACHE:
        _NC_CACHE["nc"] = build_kernel()
    return _NC_CACHE["nc"]


def kernel(char_ids, emb_table, conv_w, conv_b, trace=False):
    if trace:
        _ensure_ntff_hook()
    nc = _get_nc()
    in_maps = host_prep(char_ids, emb_table, conv_w, conv_b)
    res = run_bass_kernel_spmd(
        nc, in_maps, core_ids=list(range(NCORES)), trace=trace
    )
    # out[f, word] word-linear -> [word, f]
    outs = [res.results[jc]["out"].T for jc in range(NCORES)]
    full = np.concatenate(outs, axis=0).reshape(B, S, F).astype(np.float32)
    if trace:
        return full, res
    return full
